# revision 68
# baseline (speedup 1.0000x reference)
"""Trainium2 Bass kernel for nn_Attention (B=2, N=2048, DIM=2048, H=16, HD=128).

Sharding: 8 cores = 2 batches x 4 head-groups (4 heads each). Each core:
  - QKV projection (token-partition layout); x/coef streamed on dedicated
    DMA queues so ring-slot waits self-pace without head-of-line blocking
  - per-head RMSNorm (2 wide ACT squares + segmented DVE reduce; gammas
    folded into host-precomputed RoPE coefficients), RoPE on host-
    deinterleaved q/k channels (packed fp16, 2x DVE), PE-transpose of Q/K
    to [hd, n] (tile 15's transpose deferred into the first attention slot)
  - S^T = K^T.T @ Q^T scores (m on partitions), exp on ACT (no max needed:
    scores are O(few) for this regime), softmax sums via a 128-wide-ones
    matmul over quad-reduced exp tiles (pre-broadcast sums, 4x fewer PE
    columns), PV accumulation, normalization via reciprocal_approx_fast,
    output projection interleaved into the following chunk's ACT-bound
    head loops, fp16 partials to HBM
  - host sums the 4 head-group partials per batch.
"""

import sys

import numpy as np

sys.path.insert(0, "/opt/trn_rl_repo")

import ml_dtypes  # noqa: E402

import concourse.bass as bass  # noqa: E402
import concourse.tile as tile  # noqa: E402
from concourse import bacc  # noqa: E402
from concourse import mybir  # noqa: E402
from concourse.masks import make_identity  # noqa: E402

B, N, DIM, H, HD = 2, 2048, 2048, 16, 128
NCORES = 8
GROUPS = NCORES // B  # 4 head-groups
HPC = H // GROUPS  # 4 heads per core
CPC = HPC * HD  # 512 channels per core
EPS = 1e-5
SCALE = 1.0 / float(np.sqrt(HD))
EXP_OFF = -7.0  # keeps exp(s) in fp16 range (max observed score*scale ~16); cancels in softmax

NT = N // 128  # 16 token tiles
DT = DIM // 128  # 16 contraction tiles
NJ = N // 512  # 4 n-chunks
XG = 4  # token tiles per x-DMA generation

F32 = mybir.dt.float32
BF16 = mybir.dt.float16  # fp16: 8x finer mantissa than bf16, same PE rate
F8 = mybir.dt.float8e4
NPBF16 = np.float16
AF = mybir.ActivationFunctionType


def _emit(tc: "tile.TileContext"):
    nc = tc.nc
    xT = nc.dram_tensor("xT", [DIM, N], BF16, kind="ExternalInput")
    wqkvT = nc.dram_tensor("wqkvT", [DIM, 3 * CPC], BF16, kind="ExternalInput")
    woutT = nc.dram_tensor("woutT", [CPC, DIM], BF16, kind="ExternalInput")
    coef = nc.dram_tensor("coef", [N, 8, 2 * HD], BF16, kind="ExternalInput")
    outp = nc.dram_tensor("outp", [N, DIM], BF16, kind="ExternalOutput")

    with (
        tc.tile_pool(name="const", bufs=1) as const,
        tc.tile_pool(name="persist", bufs=1) as persist,
        # qr outlives phase 1: tile 15's transpose is deferred into phase 2
        tc.tile_pool(name="qr", bufs=3) as qrp,
    ):
        ident = const.tile([128, 128], BF16)
        make_identity(nc, ident)
        ones128 = const.tile([128, 128], BF16)
        nc.vector.memset(ones128, 1.0)
        eps_sb = const.tile([128, 1], F32)
        nc.vector.memset(eps_sb, EPS)
        expoff_sb = const.tile([128, 1], F32)
        nc.vector.memset(expoff_sb, EXP_OFF)
        # warm the ACT exp table while the lead-in is DMA-bound, so the
        # phase-2 transition doesn't pay the 1.3us table load
        warm = const.tile([1, 1], F32)
        nc.scalar.activation(out=warm, in_=eps_sb[0:1, :], func=AF.Exp)

        # persistent activations, split per 512-token chunk so phase 2 can
        # start before the whole of phase 1 has drained
        QT = [persist.tile([128, HPC, 512], BF16, tag=f"QT{j}", name=f"QT{j}") for j in range(NJ)]
        KT = [persist.tile([128, HPC, 512], BF16, tag=f"KT{j}", name=f"KT{j}") for j in range(NJ)]
        V = [persist.tile([128, 4, CPC], BF16, tag=f"V{j}", name=f"V{j}") for j in range(NJ)]
        O = [persist.tile([128, HPC, 512], BF16, tag=f"O{j}", name=f"O{j}") for j in range(NJ)]
        wout_sb = const.tile([128, HPC, DIM], BF16)
        qr_tiles = {}

        # ---------------- phase 1: QKV + rmsnorm + rope + transpose ------
        with (
            tc.tile_pool(name="wq", bufs=1) as wqp,
            tc.tile_pool(name="xs", bufs=2) as xsp,
            tc.tile_pool(name="cf", bufs=3) as cfp,
            tc.tile_pool(name="qn", bufs=1) as qnp,
            tc.tile_pool(name="scr", bufs=2) as scrp,
            tc.tile_pool(name="qkv_ps", bufs=2, space="PSUM") as qkvps,
            tc.tile_pool(name="tr_ps", bufs=2, space="PSUM") as trps,
        ):
            wq_sb = [None] * DT
            xgen = {}
            cfs = {}
            xTr = xT.rearrange("(dd p) n -> p dd n", p=128)

            def load_wq(t):
                wt = wqp.tile([128, 3 * CPC], BF16, tag=f"wq{t}", name="wt")
                (nc.sync if t % 2 == 0 else nc.scalar).dma_start(
                    out=wt, in_=wqkvT[t * 128 : (t + 1) * 128, :]
                )
                wq_sb[t] = wt

            def load_xhalf(g, half):
                # two batched DMAs per x generation (d 0-7, d 8-15), alone on
                # the sync queue: ring-slot waits self-pace the stream without
                # head-of-line-blocking other loads, and issue cost is tiny
                xb = xsp.tile([128, 8, XG * 128], BF16, tag=f"xg{half}", name="xb")
                nc.sync.dma_start(
                    out=xb,
                    in_=xTr[:, half * 8 : half * 8 + 8, g * XG * 128 : (g + 1) * XG * 128],
                )
                xgen.setdefault(g, [None, None])[half] = xb

            def load_xgen(g):
                load_xhalf(g, 0)
                load_xhalf(g, 1)

            def load_cf(i):
                cf = cfp.tile([128, 8, 2 * HD], BF16, tag="cf", name="cf")
                nc.scalar.dma_start(
                    out=cf, in_=coef[i * 128 : (i + 1) * 128, :, :]
                )
                cfs[i] = cf

            # initial loads, interleaved in PE consumption order
            load_wq(0)   # sync
            load_wq(1)   # scalar
            load_xhalf(0, 0)
            load_wq(2)
            load_wq(3)
            load_wq(5)
            load_cf(0)
            load_wq(4)
            load_wq(7)
            load_cf(1)
            for t in (6, 8):
                load_wq(t)
            load_xhalf(0, 1)
            for t in (10, 12, 14):
                load_wq(t)
            for t in (9, 11, 13, 15):
                load_wq(t)
            load_xgen(1)
            nc.scalar.dma_start(
                out=wout_sb, in_=woutT.rearrange("(h p) d -> p h d", p=128)
            )

            def transposes(i):
                qr = qr_tiles[i % 3]
                for qk in range(2):
                    trp = trps.tile([128, CPC], BF16, name="trp")
                    for h in range(HPC):
                        hsl = slice(h * HD, (h + 1) * HD)
                        nc.tensor.transpose(
                            trp[:, hsl], qr[:, qk, hsl], ident
                        )
                    tgt = (QT if qk == 0 else KT)[i // 4]
                    dst = tgt[:, :, (i % 4) * 128 : (i % 4 + 1) * 128]
                    nc.vector.tensor_copy(
                        out=dst,
                        in_=trp.rearrange("p (h n) -> p h n", h=HPC),
                    )

            for i in range(NT):
                nsl = slice(i * 128, (i + 1) * 128)
                g = i // XG
                if i % XG == 0 and g + 2 < NT // XG:
                    load_xgen(g + 2)
                if i + 2 < NT:
                    load_cf(i + 2)
                ps = qkvps.tile([128, 3, CPC], F32, name="ps")
                xsl = slice((i % XG) * 128, (i % XG + 1) * 128)
                for d in range(DT):
                    for c in range(3):
                        nc.tensor.matmul(
                            ps[:, c, :],
                            lhsT=xgen[g][d // 8][:, d % 8, xsl],
                            rhs=wq_sb[d][:, c * CPC : (c + 1) * CPC],
                            start=(d == 0),
                            stop=(d == DT - 1),
                        )

                # transposes lag 2 tiles so the PE never waits on the
                # rmsnorm/rope chain of the tile being transposed
                if i > 1:
                    transposes(i - 2)

                # V straight to SBUF (bf16)
                nc.vector.tensor_copy(out=V[i // 4][:, i % 4, :], in_=ps[:, 2, :])

                # rmsnorm: 2 wide squares on ACT + one segmented DVE reduce
                # (replaces 8 accum-squares: far less ACT serial time)
                ssq = scrp.tile([128, 8], F32, tag="ssq")
                sq = scrp.tile([128, 2, CPC], BF16, tag="sq", bufs=1)
                for qk in range(2):
                    nc.scalar.activation(
                        out=sq[:, qk, :], in_=ps[:, qk, :], func=AF.Square
                    )
                nc.vector.tensor_reduce(
                    out=ssq,
                    in_=sq.rearrange("p a (s hd) -> p (a s) hd", hd=HD),
                    axis=mybir.AxisListType.X,
                    op=mybir.AluOpType.add,
                )
                rstd = scrp.tile([128, 8], F32, tag="rstd")
                nc.scalar.activation(
                    rstd, ssq, AF.Sqrt, bias=eps_sb, scale=1.0 / HD
                )
                nc.vector.reciprocal(rstd, rstd)

                qn = qnp.tile([128, 2, CPC], BF16, name="qn")
                for qk in range(2):
                    for h in range(HPC):
                        hsl = slice(h * HD, (h + 1) * HD)
                        nc.vector.tensor_scalar_mul(
                            out=qn[:, qk, hsl],
                            in0=ps[:, qk, hsl],
                            scalar1=rstd[:, qk * HPC + h : qk * HPC + h + 1],
                        )

                # rope; q/k channels are host-permuted to [evens|odds] per
                # head, so every operand here is packed fp16 (2x DVE rate)
                cf = cfs.pop(i)
                qr = qrp.tile([128, 2, CPC], BF16, name="qr")
                qr_tiles[i % 3] = qr
                for qk in range(2):
                    base = qk * 4
                    dq = qn[:, qk, :].rearrange("p (h z c) -> p h z c", z=2, c=HD // 2)
                    x0 = dq[:, :, 0, :]
                    x1 = dq[:, :, 1, :]
                    rot = qr[:, qk, :].rearrange("p (h z c) -> p h z c", z=2, c=HD // 2)

                    def cf3(k):
                        return cf[:, base + k, :].rearrange("p (h c) -> p h c", c=HD // 2)

                    ta = scrp.tile([128, HPC, HD // 2], BF16, tag="ta", bufs=1)
                    tb = scrp.tile([128, HPC, HD // 2], BF16, tag="tb", bufs=1)
                    nc.vector.tensor_mul(ta, x0, cf3(0))
                    nc.vector.tensor_mul(tb, x1, cf3(1))
                    nc.vector.tensor_sub(rot[:, :, 0, :], ta, tb)
                    tc2 = scrp.tile([128, HPC, HD // 2], BF16, tag="tc2", bufs=1)
                    td = scrp.tile([128, HPC, HD // 2], BF16, tag="td", bufs=1)
                    nc.vector.tensor_mul(tc2, x0, cf3(2))
                    nc.vector.tensor_mul(td, x1, cf3(3))
                    nc.vector.tensor_add(rot[:, :, 1, :], tc2, td)

            transposes(NT - 2)
            # tile 15's transpose is deferred into phase 2 slot (0,0): its
            # rmsnorm/rope chain would otherwise stall the PE ~9us here

        # ------------- phase 2+3: attention + output projection ----------
        with (
            tc.tile_pool(name="ps2", bufs=3, space="PSUM") as sps,
            tc.tile_pool(name="op_ps", bufs=2, space="PSUM") as opsp,
            tc.tile_pool(name="o_ps", bufs=2, space="PSUM") as ops_,
            tc.tile_pool(name="sum_ps", bufs=1, space="PSUM") as sums_,
            tc.tile_pool(name="es", bufs=2) as esp,
            tc.tile_pool(name="pa", bufs=2) as pap,
            tc.tile_pool(name="qd", bufs=2) as qdp,
            tc.tile_pool(name="invsb", bufs=2) as invsbp,
            tc.tile_pool(name="ob", bufs=4) as obp,
        ):
            # outproj emitted as fine-grained items interleaved into the
            # (ACT-bound) m-loops of the following chunk's head slots, so
            # the PE's per-step slack absorbs it instead of serializing
            op_state = {}

            def emit_op_item(item):
                if item[0] == "mm":
                    _, jj, it, dch, hh = item
                    if hh == 0:
                        op_state["ps"] = opsp.tile(
                            [128, 512], F32, tag="op", name="op_ps"
                        )
                    nc.tensor.matmul(
                        op_state["ps"],
                        lhsT=O[jj][:, hh, it * 128 : (it + 1) * 128],
                        rhs=wout_sb[:, hh, dch * 512 : (dch + 1) * 512],
                        start=(hh == 0),
                        stop=(hh == HPC - 1),
                    )
                else:
                    _, jj, it, dch = item
                    nsl = slice((4 * jj + it) * 128, (4 * jj + it + 1) * 128)
                    ob = obp.tile([128, 512], BF16, tag="ob", name="ob")
                    nc.vector.tensor_copy(out=ob, in_=op_state["ps"])
                    (nc.sync if dch % 2 == 0 else nc.scalar).dma_start(
                        out=outp[nsl, dch * 512 : (dch + 1) * 512], in_=ob
                    )

            def push_op_items(jj):
                for it in range(4):
                    for dch in range(4):
                        for hh in range(HPC):
                            opq.append(("mm", jj, it, dch, hh))
                        opq.append(("cp", jj, it, dch))

            # op items avoid m=5/9/13 where the sums matmuls land
            OPS_AT_M = {3: 2, 4: 2, 6: 2, 7: 2, 8: 2, 10: 2, 11: 2, 12: 2,
                        14: 2, 15: 2}

            def emit_tr15(qk):
                i = NT - 1
                qr = qr_tiles[i % 3]
                if True:
                    trp = opsp.tile([128, CPC], BF16, tag="op", name="trp")
                    for hh in range(HPC):
                        hsl = slice(hh * HD, (hh + 1) * HD)
                        nc.tensor.transpose(trp[:, hsl], qr[:, qk, hsl], ident)
                    tgt = (QT if qk == 0 else KT)[i // 4]
                    nc.vector.tensor_copy(
                        out=tgt[:, :, (i % 4) * 128 : (i % 4 + 1) * 128],
                        in_=trp.rearrange("p (h n) -> p h n", h=HPC),
                    )

            opq = []
            pend_tail = None
            for j in range(NJ):
                for h in range(HPC):
                    o_ps = ops_.tile([128, 512], F32, tag="o", name="o_ps")
                    sum_ps = sums_.tile([128, 512], F32, tag="sum", name="sum_ps")
                    ess = {}
                    pas = {}
                    qd8s = {}

                    def pv(m, o_ps=o_ps, ess=ess, h=h):
                        nc.tensor.matmul(
                            o_ps,
                            lhsT=V[m // 4][:, m % 4, h * HD : (h + 1) * HD],
                            rhs=ess[m],
                            start=(m == 0),
                            stop=(m == NT - 1),
                        )

                    def sums_mm(q, sum_ps=sum_ps, qd8s=qd8s):
                        # 128-wide ones lhsT: same PE cost (free-size only),
                        # but the sums land pre-broadcast on all partitions
                        nc.tensor.matmul(
                            sum_ps,
                            lhsT=ones128,
                            rhs=qd8s[q],
                            start=(q == 0),
                            stop=(q == 3),
                        )

                    def tail(j=j, h=h, o_ps=o_ps, sum_ps=sum_ps, pv=pv, sums_mm=sums_mm):
                        # ACT-gated slot tail, emitted inside the NEXT slot's
                        # m-loop so the PE never drains waiting for exp(14/15)
                        pv(NT - 2)
                        pv(NT - 1)
                        sums_mm(3)
                        inv = invsbp.tile([128, 512], F32, tag="invsb", name="inv")
                        nc.vector.reciprocal_approx_fast(out=inv, in_=sum_ps)
                        nc.vector.tensor_mul(O[j][:, h, :], o_ps, inv)
                        if h == HPC - 1:
                            push_op_items(j)

                    for m in range(NT):
                        s_ps = sps.tile([128, 512], F32, tag="s", name="s_ps")
                        nc.tensor.matmul(
                            s_ps,
                            lhsT=KT[m // 4][:, h, (m % 4) * 128 : (m % 4 + 1) * 128],
                            rhs=QT[j][:, h, :],
                            start=True,
                            stop=True,
                        )
                        es = esp.tile([128, 512], BF16, tag=f"es{m % 4}", name="es")
                        nc.scalar.activation(
                            es, s_ps, AF.Exp, scale=SCALE, bias=expoff_sb
                        )
                        ess[m] = es
                        if m == 1 and pend_tail is not None:
                            pend_tail()
                            pend_tail = None
                        # quad-reduce exp tiles on DVE: 4x fewer sum columns on PE
                        if m % 2 == 1:
                            pa = pap.tile([128, 512], BF16, tag=f"pa{(m // 2) % 2}", name="pa")
                            nc.vector.tensor_add(pa, ess[m - 1], ess[m])
                            pas[m // 2] = pa
                        if m % 4 == 3:
                            q = m // 4
                            qd8s[q] = qdp.tile([128, 512], BF16, tag="qd", name="qd")
                            nc.vector.tensor_add(
                                qd8s[q], pas[m // 2 - 1], pas[m // 2]
                            )
                        if m in (5, 9, 13):
                            sums_mm((m - 5) // 4)
                        if m >= 2:
                            pv(m - 2)
                            for _ in range(OPS_AT_M.get(m, 0)):
                                if opq:
                                    emit_op_item(opq.pop(0))
                            if j == 0 and h == 0 and m in (12, 13):
                                emit_tr15(m - 12)
                    pend_tail = tail

            pend_tail()
            while opq:
                emit_op_item(opq.pop(0))


_NC = None


def _get_nc():
    global _NC
    if _NC is None:
        nc = bacc.Bacc()
        with tile.TileContext(nc) as tc:
            _emit(tc)
        if not nc.is_finalized():
            nc.finalize()
        _NC = nc
    return _NC


def _deint(W):
    # reorder each head's output channels to [evens | odds] so rope pairs
    # (2i, 2i+1) become (i, i+64): packed DVE access on-device; scores are
    # invariant to any per-head channel permutation applied to both q and k
    W = W.reshape(HPC, HD, DIM)
    W = np.concatenate([W[:, 0::2, :], W[:, 1::2, :]], axis=1)
    return W.reshape(HPC * HD, DIM)


def _prep_core(x, Wqkv, q_gamma, k_gamma, Wout, cos, sin, b, hg):
    hsl = slice(hg * CPC, (hg + 1) * CPC)
    Wq = _deint(Wqkv[0 * H * HD : 1 * H * HD][hsl])
    Wk = _deint(Wqkv[1 * H * HD : 2 * H * HD][hsl])
    Wv = Wqkv[2 * H * HD : 3 * H * HD][hsl]
    wqkvT = np.ascontiguousarray(np.concatenate([Wq, Wk, Wv], 0).T)
    woutT = np.ascontiguousarray(Wout[:, hsl].T)

    def c4(a):  # [N, 64] -> [N, 256] tiled over the 4 heads
        return np.tile(a, (1, HPC))

    qe, qo = q_gamma[0::2], q_gamma[1::2]
    ke, ko = k_gamma[0::2], k_gamma[1::2]
    cb, sb = cos[b], sin[b]  # [N, 64]
    coef = np.stack(
        [
            c4(cb * qe), c4(sb * qo), c4(sb * qe), c4(cb * qo),
            c4(cb * ke), c4(sb * ko), c4(sb * ke), c4(cb * ko),
        ],
        axis=1,
    ).astype(np.float16)  # [N, 8, 256]
    return {
        "xT": np.ascontiguousarray(x[b].T).astype(NPBF16),
        "wqkvT": wqkvT.astype(NPBF16),
        "woutT": woutT.astype(NPBF16),
        "coef": np.ascontiguousarray(coef),
    }


def prep_in_maps(x, Wqkv, q_gamma, k_gamma, Wout, freqs):
    x = np.asarray(x, np.float32)
    Wqkv = np.asarray(Wqkv, np.float32)
    Wout = np.asarray(Wout, np.float32)
    q_gamma = np.asarray(q_gamma, np.float32)
    k_gamma = np.asarray(k_gamma, np.float32)
    freqs = np.asarray(freqs, np.float32)
    cos = freqs[..., 0]
    sin = freqs[..., 1]
    return [
        _prep_core(x, Wqkv, q_gamma, k_gamma, Wout, cos, sin, c // GROUPS, c % GROUPS)
        for c in range(NCORES)
    ]


def gather(parts):
    out = np.empty((B, N, DIM), np.float32)
    for b in range(B):
        acc = parts[b * GROUPS].astype(np.float32)
        for g in range(1, GROUPS):
            acc = acc + parts[b * GROUPS + g]
        out[b] = acc
    return out


def kernel(x, Wqkv, q_gamma, k_gamma, Wout, freqs):
    from concourse.bass_utils import run_bass_kernel_spmd

    nc = _get_nc()
    in_maps = prep_in_maps(x, Wqkv, q_gamma, k_gamma, Wout, freqs)
    res = run_bass_kernel_spmd(nc, in_maps, list(range(NCORES)))
    parts = [res.results[c]["outp"] for c in range(NCORES)]
    return gather(parts)
